# revision 5
# baseline (speedup 1.0000x reference)
"""Self-attention block (LayerNorm + QKV + QK-RMSNorm + softmax attention +
output projection) on 8 TRN2 NeuronCores.

Sharding: core c handles batch b = c//4 and head-group g = c%4 (4 of the 16
heads).  Each core computes a partial output projection for its 4 heads; the
host sums the 4 partials per batch.

Design notes:
  - all matmul operands bf16 (psum f32); inputs shipped bf16, x pre-transposed
    on the host so no on-device transposes are needed
  - LN mean-removal is folded into host-centered weight rows
    ((x-mu) @ W^T == x @ Wc^T with Wc rows centered over the input dim);
    only rstd is computed on device (row-sum matmuls against a ones column),
    and it only scales V
  - q/k rms-norm scales via "broadcast blockdiag" matmuls: the stationary
    replicates 1/gamma^2 across all 128 output columns so each (head, pos)
    squared norm lands on every row of its head's range in one matmul
  - attention is software-pipelined ACROSS (q-chunk, head-pair) chunks: the
    exp stream on the scalar engine is the bottleneck, so each chunk's first
    two score/exp pairs are emitted in the previous chunk's empty score
    slots, scores are emitted ahead of AV within each iteration, softmax
    denominators ride along as a ones-column in the AV stationary, and the
    previous q-chunk's output projection is spread one unit per odd
    key-block to fill the PE's exp-latency slack
"""

import os

import numpy as np

import concourse.bacc as bacc
import concourse.bass as bass
import concourse.mybir as mybir
import concourse.tile as tile
from concourse import bass_utils

try:
    import axon_profile_shim

    axon_profile_shim.install()
except Exception:
    pass

B, N, D = 2, 2048, 1024
H_TOT, DH = 16, 64
HPC = 4  # heads per core
DPC = HPC * DH  # 256 head-dims per core
P = 128
NT = N // P  # 16 seq tiles
KC = D // P  # 8 contraction chunks
NC4 = N // 512  # 4 n-chunks of 512
LN_EPS = 1e-5

F32 = mybir.dt.float32
BF = mybir.dt.bfloat16
AF = mybir.ActivationFunctionType

_CACHE = {}
DEBUG = bool(int(os.environ.get("KERNEL_DEBUG", "0")))


def build():
    nc = bacc.Bacc("TRN2", target_bir_lowering=False, debug=False, num_devices=8)

    xT_d = nc.dram_tensor("xT", [D, N], BF, kind="ExternalInput")
    wq_d = nc.dram_tensor("wqT", [D, DPC], BF, kind="ExternalInput")
    wk_d = nc.dram_tensor("wkT", [D, DPC], BF, kind="ExternalInput")
    wv_d = nc.dram_tensor("wvT", [D, DPC], BF, kind="ExternalInput")
    wo_d = nc.dram_tensor("woT", [DPC, D], BF, kind="ExternalInput")
    bo_d = nc.dram_tensor("bo_bc", [P, D], F32, kind="ExternalInput")
    bd_d = nc.dram_tensor("bd", [P, 2, P], BF, kind="ExternalInput")
    out_d = nc.dram_tensor("out", [N, D], F32, kind="ExternalOutput")
    if DEBUG:
        dbg = {
            nm: nc.dram_tensor(nm, shp, dt, kind="ExternalOutput")
            for nm, (shp, dt) in {
                "dbg_xcT": ([P, KC * N], BF),
                "dbg_qnT": ([P, 2 * N], BF),
                "dbg_knT": ([P, 2 * N], BF),
                "dbg_vsb": ([P, NT * HPC * P], BF),
                "dbg_mrg": ([P, 2 * N], BF),
                "dbg_rstd": ([P, NT], F32),
            }.items()
        }

    with tile.TileContext(nc) as tc:
        with tc.tile_pool(name="outer", bufs=1) as op0:
            xsT = op0.tile([P, KC, N], BF, tag="xsT")
            qnT = op0.tile([P, 2, N], BF, tag="qnT")
            knT = op0.tile([P, 2, N], BF, tag="knT")
            vsb = op0.tile([P, NT, HPC, P], BF, tag="vsb")
            mrg = op0.tile([P, 2, N], BF, tag="mrg")
            rstd_all = op0.tile([P, NT], F32, tag="rstd")
            wq = op0.tile([P, KC, DPC], BF, tag="wq")
            wk = op0.tile([P, KC, DPC], BF, tag="wk")
            wv = op0.tile([P, KC, DPC], BF, tag="wv")
            wo = op0.tile([P, 2, D], BF, tag="wo")
            bo = op0.tile([P, D], F32, tag="bo")
            bd = op0.tile([P, 2, P], BF, tag="bd")

            # denominator ones columns: even-head slots col 64, odd col 0
            for h in range(HPC):
                col = 64 if h % 2 == 0 else 0
                nc.vector.memset(vsb[:, :, h, col : col + 1], 1.0)

            def emit_weight_dmas():
                nc.sync.dma_start(wq, wq_d.ap().rearrange("(c p) m -> p c m", p=P))
                nc.scalar.dma_start(wk, wk_d.ap().rearrange("(c p) m -> p c m", p=P))
                nc.sync.dma_start(wv, wv_d.ap().rearrange("(c p) m -> p c m", p=P))
                nc.scalar.dma_start(wo, wo_d.ap().rearrange("(c p) m -> p c m", p=P))
                nc.sync.dma_start(bo, bo_d.ap())
                nc.scalar.dma_start(bd, bd_d.ap())

            # ---- phase 1: xT streams in by 512-col span; LN mean removal is
            # folded into host-centered weights, so projections consume raw
            # xT.  Per span: row-sum/rowsq-sum matmuls give mu and var for
            # rstd (v scaling only); k/v/q projections + rms-norm scales.
            with (
                tc.tile_pool(name="sqxp", bufs=2) as sqxp,
                tc.tile_pool(name="statp", bufs=2) as statp,
                tc.tile_pool(name="sqp", bufs=3) as sqp,
                tc.tile_pool(name="sdp", bufs=3) as sdp,
                tc.tile_pool(name="rrp", bufs=3) as rrp,
                tc.tile_pool(name="onep", bufs=1) as onep,
                tc.tile_pool(name="ps_raw", bufs=3, space="PSUM") as ps_raw,
                tc.tile_pool(name="ps_n2", bufs=2, space="PSUM") as ps_n2,
                tc.tile_pool(name="ps_v", bufs=2, space="PSUM") as ps_v,
                tc.tile_pool(name="ps_st", bufs=1, space="PSUM") as ps_st,
            ):
                onesD = onep.tile([P, 1], BF)
                nc.vector.memset(onesD, 1.0 / D)
                eps_t = onep.tile([1, 1], F32)
                nc.vector.memset(eps_t, LN_EPS)

                def emit_unit(w_sb, pt, ncn):
                    cs = slice(ncn * 512, (ncn + 1) * 512)
                    raw = ps_raw.tile([P, 512], F32, tag="raw")
                    for dc in range(KC):
                        nc.tensor.matmul(
                            raw,
                            w_sb[:, dc, pt * P : (pt + 1) * P],
                            xsT[:, dc, cs],
                            start=(dc == 0),
                            stop=(dc == KC - 1),
                        )
                    sq = sqp.tile([P, 512], BF, tag="sq")
                    nc.scalar.activation(sq, raw, AF.Square)
                    return raw, sq, cs

                def emit_norm_tail(raw, sq, cs, bd_idx, sqrt_scale, dstT, pt):
                    n2b = ps_n2.tile([P, 512], F32, tag="n2b")
                    nc.tensor.matmul(
                        n2b, bd[:, bd_idx, :], sq, start=True, stop=True
                    )
                    sdt = sdp.tile([P, 512], F32, tag="sdt")
                    nc.scalar.activation(sdt, n2b, AF.Sqrt, scale=sqrt_scale)
                    rr = rrp.tile([P, 512], F32, tag="rr")
                    nc.vector.reciprocal_approx_fast(rr, sdt)
                    nc.vector.tensor_mul(dstT[:, pt, cs], raw, rr)

                def emit_v(st):
                    psv = ps_v.tile([P, DPC], F32, tag="psv")
                    for dc in range(KC):
                        nc.tensor.matmul(
                            psv,
                            xsT[:, dc, st * P : (st + 1) * P],
                            wv[:, dc, :],
                            start=(dc == 0),
                            stop=(dc == KC - 1),
                        )
                    pv = psv.rearrange("p (h e d) -> p h e d", h=2, e=2)
                    nc.vector.tensor_scalar_mul(
                        vsb[:, st, 0:4:2, 0:64],
                        pv[:, :, 0],
                        rstd_all[:, st : st + 1],
                    )
                    nc.vector.tensor_scalar_mul(
                        vsb[:, st, 1:4:2, 64:128],
                        pv[:, :, 1],
                        rstd_all[:, st : st + 1],
                    )

                pend = [None]

                def flush_tail():
                    if pend[0] is not None:
                        emit_norm_tail(*pend[0])
                        pend[0] = None

                # prefetch all xT chunk DMAs up front (xsT is persistent, so
                # the queues stream ahead of the span compute); weights go
                # after span 0's chunks
                for s in range(NC4):
                    cs = slice(s * 512, (s + 1) * 512)
                    for dc in range(KC):
                        qe = (nc.sync, nc.scalar, nc.gpsimd)[dc % 3]
                        qe.dma_start(
                            xsT[:, dc, cs],
                            xT_d.ap()[dc * P : (dc + 1) * P, cs],
                        )
                    if s == 0:
                        emit_weight_dmas()

                for s in range(NC4):
                    cs = slice(s * 512, (s + 1) * 512)
                    # squared xT for the variance row-sums
                    sqx = sqxp.tile([P, KC, 512], BF, tag="sqx")
                    for dc in range(KC):
                        nc.vector.tensor_mul(
                            sqx[:, dc], xsT[:, dc, cs], xsT[:, dc, cs]
                        )
                    # mu row: (1/D) * ones^T @ xT
                    mups = ps_st.tile([1, 512], F32, tag="strow", name="mups")
                    for dc in range(KC):
                        nc.tensor.matmul(
                            mups,
                            onesD,
                            xsT[:, dc, cs],
                            start=(dc == 0),
                            stop=(dc == KC - 1),
                        )
                    raw, sq_, cs_ = emit_unit(wk, 0, s)
                    flush_tail()
                    pend[0] = (raw, sq_, cs_, 1, 1.0 / 64.0, knT, 0)
                    raw, sq_, cs_ = emit_unit(wk, 1, s)
                    flush_tail()
                    pend[0] = (raw, sq_, cs_, 1, 1.0 / 64.0, knT, 1)
                    # E[x^2] row
                    n2xps = ps_st.tile([1, 512], F32, tag="strow", name="n2xps")
                    for dc in range(KC):
                        nc.tensor.matmul(
                            n2xps,
                            onesD,
                            sqx[:, dc],
                            start=(dc == 0),
                            stop=(dc == KC - 1),
                        )
                    flush_tail()
                    # rstd for this span: 1/sqrt(E[x^2] - mu^2 + eps), shipped
                    # to natural [P, tile] layout via an SBUF-shuffle DMA
                    musb = statp.tile([1, 512], F32, tag="musb")
                    nc.vector.tensor_copy(musb, mups)
                    sqmu = statp.tile([1, 512], F32, tag="sqmu")
                    nc.vector.tensor_mul(sqmu, musb, musb)
                    varT = statp.tile([1, 512], F32, tag="varT")
                    nc.vector.tensor_sub(varT, n2xps, sqmu)
                    sdT = statp.tile([1, 512], F32, tag="sdT")
                    nc.scalar.activation(sdT, varT, AF.Sqrt, bias=eps_t)
                    rsT = statp.tile([1, 512], F32, tag="rsT")
                    nc.vector.reciprocal_approx_fast(rsT, sdT)
                    for j in range(4):
                        qe = nc.sync if j % 2 == 0 else nc.scalar
                        qe.dma_start(
                            rstd_all[:, 4 * s + j : 4 * s + j + 1],
                            rsT[:, j * P : (j + 1) * P],
                        )
                    for st in range(4 * s, 4 * s + 4):
                        emit_v(st)
                    raw, sq_, cs_ = emit_unit(wq, 0, s)
                    flush_tail()
                    pend[0] = (raw, sq_, cs_, 0, 1.0, qnT, 0)
                    raw, sq_, cs_ = emit_unit(wq, 1, s)
                    flush_tail()
                    pend[0] = (raw, sq_, cs_, 0, 1.0, qnT, 1)
                flush_tail()

            if DEBUG:
                nc.sync.dma_start(
                    dbg["dbg_xcT"].ap(), xsT.rearrange("p c n -> p (c n)")
                )
                nc.sync.dma_start(dbg["dbg_rstd"].ap(), rstd_all)

            if DEBUG:
                nc.sync.dma_start(
                    dbg["dbg_qnT"].ap(), qnT.rearrange("p c n -> p (c n)")
                )
                nc.sync.dma_start(
                    dbg["dbg_knT"].ap(), knT.rearrange("p c n -> p (c n)")
                )
                nc.sync.dma_start(
                    dbg["dbg_vsb"].ap(), vsb.rearrange("p a b c -> p (a b c)")
                )

            # ---- phase 2: attention + interleaved output projection ----
            with (
                tc.tile_pool(name="expp", bufs=6) as ep,
                tc.tile_pool(name="rec", bufs=4) as rp,
                tc.tile_pool(name="bcp", bufs=4) as bcp,
                tc.tile_pool(name="outp", bufs=4) as outp,
                tc.tile_pool(name="ps_sim", bufs=2, space="PSUM") as ps_sim,
                tc.tile_pool(name="ps_av", bufs=4, space="PSUM") as ps_av,
            ):
                def emit_scores(hp, qs, kt, pss):
                    ks = slice(kt * P, (kt + 1) * P)
                    nc.tensor.matmul(
                        pss[:, 0:512],
                        knT[0:64, hp, ks],
                        qnT[0:64, hp, qs],
                        start=True, stop=True,
                        tile_position=(0, 0),
                    )
                    nc.tensor.matmul(
                        pss[:, 512:1024],
                        knT[64:128, hp, ks],
                        qnT[64:128, hp, qs],
                        start=True, stop=True,
                        tile_position=(64, 0),
                    )

                pending_outproj = [None]
                CHUNKS = [(qc, hp) for qc in range(NC4) for hp in range(2)]
                carry_exs = [None]

                for ci, (qc, hp) in enumerate(CHUNKS):
                    qs = slice(qc * 512, (qc + 1) * 512)
                    if True:
                        h0, h1 = 2 * hp, 2 * hp + 1
                        # scores run 2 kt ahead of AV; the first two pairs of
                        # each chunk are emitted during the previous chunk's
                        # empty score slots (kt14/kt15) so the exp stream
                        # never drains at chunk boundaries
                        if carry_exs[0] is None:
                            exs = [None] * NT
                            for kt in range(2):
                                pss = ps_sim.tile(
                                    [P, 1024], F32, tag="pss", name=f"pssp{kt}"
                                )
                                emit_scores(hp, qs, kt, pss)
                                exs[kt] = ep.tile(
                                    [P, 1024], BF, tag="ex", name=f"exp{kt}"
                                )
                                nc.scalar.activation(exs[kt], pss, AF.Exp)
                        else:
                            exs = carry_exs[0]
                        nxt = CHUNKS[ci + 1] if ci + 1 < len(CHUNKS) else None
                        nexs = [None] * NT if nxt else None
                        pv0 = ps_av.tile([P, 512], F32, tag="pav")
                        pv1 = ps_av.tile([P, 512], F32, tag="pav")
                        for kt in range(NT):
                            # emit scores before AV: both are gated on
                            # exp(kt), but AV can additionally stall on the
                            # pv ring at chunk start without blocking ACT
                            if kt + 2 < NT:
                                pss = ps_sim.tile(
                                    [P, 1024], F32, tag="pss", name=f"pssn{kt}"
                                )
                                emit_scores(hp, qs, kt + 2, pss)
                                nx = ep.tile([P, 1024], BF, tag="ex", name=f"exn{kt}")
                                nc.scalar.activation(nx, pss, AF.Exp)
                                exs[kt + 2] = nx
                            elif nxt is not None:
                                kn = kt + 2 - NT
                                nqc, nhp = nxt
                                nqs = slice(nqc * 512, (nqc + 1) * 512)
                                pss = ps_sim.tile(
                                    [P, 1024], F32, tag="pss", name=f"pssx{kn}"
                                )
                                emit_scores(nhp, nqs, kn, pss)
                                nx = ep.tile([P, 1024], BF, tag="ex", name=f"exx{kn}")
                                nc.scalar.activation(nx, pss, AF.Exp)
                                nexs[kn] = nx
                            ex = exs[kt]
                            nc.tensor.matmul(
                                pv0,
                                vsb[:, kt, h0, :],
                                ex[:, 0:512],
                                start=(kt == 0),
                                stop=(kt == NT - 1),
                            )
                            nc.tensor.matmul(
                                pv1,
                                vsb[:, kt, h1, :],
                                ex[:, 512:1024],
                                start=(kt == 0),
                                stop=(kt == NT - 1),
                            )
                            # previous qc's output projection, spread one
                            # (st, osl) unit per odd kt so the PE matmuls
                            # fill the per-kt exp-latency slack
                            if hp == 0 and kt % 2 == 1 and pending_outproj[0]:
                                pending_outproj[0].pop(0)()
                                if not pending_outproj[0]:
                                    pending_outproj[0] = None
                        carry_exs[0] = nexs
                        # merge: mrg = attn_out * 1/den
                        # pv0 (even head): outs rows 0:64, den row 64
                        # pv1 (odd head):  outs rows 64:128, den row 0
                        r0 = rp.tile([P, 512], F32, tag="r0")
                        nc.vector.reciprocal_approx_fast(r0, pv0)
                        # gpsimd broadcast can only source partition 0: shift
                        # the even-head denominator row down.  SWDGE queue —
                        # the HWDGE queues are busy with output writes
                        nc.gpsimd.dma_start(r0[0:1], r0[64:65])
                        bc0 = bcp.tile([P, 512], F32, tag="bc0")
                        nc.gpsimd.partition_broadcast(bc0, r0[0:1])
                        nc.vector.tensor_mul(
                            mrg[0:64, hp, qs], pv0[0:64], bc0[0:64]
                        )
                        r1 = rp.tile([P, 512], F32, tag="r1")
                        nc.vector.reciprocal_approx_fast(r1[0:1], pv1[0:1])
                        bc1 = bcp.tile([P, 512], F32, tag="bc1")
                        nc.gpsimd.partition_broadcast(bc1, r1[0:1])
                        nc.vector.tensor_mul(
                            mrg[64:128, hp, qs], pv1[64:128], bc1[64:128]
                        )

                    # output projection for this qc (both head-pairs merged);
                    # deferred + spread into the next chunk's kt loop
                    def make_outproj_unit(st, ncn):
                        def emit():
                            ss = slice(st * P, (st + 1) * P)
                            osl = slice(ncn * 512, (ncn + 1) * 512)
                            psf = ps_av.tile(
                                [P, 512], F32, tag="pav", name=f"psf{st}_{ncn}"
                            )
                            for pt in range(2):
                                nc.tensor.matmul(
                                    psf,
                                    mrg[:, pt, ss],
                                    wo[:, pt, osl],
                                    start=(pt == 0),
                                    stop=(pt == 1),
                                )
                            ot = outp.tile(
                                [P, 512], F32, tag="ot", name=f"ot{st}_{ncn}"
                            )
                            nc.vector.tensor_add(ot, psf, bo[:, osl])
                            qe = nc.sync if (st + ncn) % 2 == 0 else nc.scalar
                            qe.dma_start(out_d.ap()[ss, osl], ot)

                        return emit

                    if hp == 1:
                        pending_outproj[0] = [
                            make_outproj_unit(qc * 4 + sti, ncn)
                            for sti in range(4)
                            for ncn in range(2)
                        ]
                for u in pending_outproj[0]:
                    u()

            if DEBUG:
                nc.sync.dma_start(
                    dbg["dbg_mrg"].ap(), mrg.rearrange("p c n -> p (c n)")
                )

    nc.compile()
    return nc


def _prep_core_inputs(inputs, c):
    import ml_dtypes

    b, g = c // 4, c % 4
    S = slice(DPC * g, DPC * (g + 1))
    x = np.ascontiguousarray(np.asarray(inputs["x"], np.float32)[b])
    lng = np.asarray(inputs["ln_gamma"], np.float32)
    lnb = np.asarray(inputs["ln_beta"], np.float32)
    qg = np.asarray(inputs["q_gamma"], np.float32)
    kg = np.asarray(inputs["k_gamma"], np.float32)
    if np.abs(lnb).max() > 0:
        raise NotImplementedError("nonzero ln_beta not supported by this kernel")
    bf16 = ml_dtypes.bfloat16
    w_q = np.asarray(inputs["w_q"], np.float32)[S] * lng[None, :]
    w_k = np.asarray(inputs["w_k"], np.float32)[S] * lng[None, :]
    w_v = np.asarray(inputs["w_v"], np.float32)[S] * lng[None, :]
    w_q = w_q * np.tile(qg, HPC)[:, None]
    w_k = w_k * np.tile(kg, HPC)[:, None]
    # LN mean-removal folded into the weights: (x - mu) @ W^T == x @ Wc^T
    # when each weight row is centered over the input dimension
    w_q = w_q - w_q.mean(axis=1, keepdims=True)
    w_k = w_k - w_k.mean(axis=1, keepdims=True)
    w_v = w_v - w_v.mean(axis=1, keepdims=True)
    w_o = np.asarray(inputs["w_o"], np.float32)[:, S]
    b_o = np.asarray(inputs["b_o"], np.float32)
    bo_bc = np.tile((b_o if g == 0 else np.zeros_like(b_o))[None, :], (P, 1))

    # broadcast blockdiag weights: bd[p, j, m] = 1/gamma[p%64]^2 when
    # partition p and output row m fall in the same head half; the resulting
    # [128, 512] matmul output carries each head's squared norm on every row
    # of that head's range (broadcast fused into the reduction).
    bd = np.zeros((P, 2, P), np.float32)
    for j, g_ in ((0, qg), (1, kg)):
        inv = 1.0 / np.maximum(g_**2, 1e-30)
        bd[0:64, j, 0:64] = inv[:, None]
        bd[64:128, j, 64:128] = inv[:, None]

    return {
        "xT": np.ascontiguousarray(x.T).astype(bf16),
        "wqT": np.ascontiguousarray(w_q.T).astype(bf16),
        "wkT": np.ascontiguousarray(w_k.T).astype(bf16),
        "wvT": np.ascontiguousarray(w_v.T).astype(bf16),
        "woT": np.ascontiguousarray(w_o.T).astype(bf16),
        "bo_bc": bo_bc,
        "bd": bd.astype(bf16),
    }


def kernel(**inputs):
    if "nc" not in _CACHE:
        _CACHE["nc"] = build()
    nc = _CACHE["nc"]
    in_maps = [_prep_core_inputs(inputs, c) for c in range(8)]
    res = bass_utils.run_bass_kernel_spmd(
        nc,
        in_maps,
        core_ids=list(range(8)),
        trace=bool(int(os.environ.get("KERNEL_TRACE", "0"))),
    )
    _CACHE["last_result"] = res
    out = np.zeros((B, N, D), np.float32)
    for c in range(8):
        out[c // 4] += res.results[c]["out"]
    return out


# revision 6
# speedup vs baseline: 1.1805x; 1.1805x over previous
"""Self-attention block (LayerNorm + QKV + QK-RMSNorm + softmax attention +
output projection) on 8 TRN2 NeuronCores.

Sharding: core c handles batch b = c//4 and head-group g = c%4 (4 of the 16
heads).  Each core computes a partial output projection for its 4 heads; the
host sums the 4 partials per batch.

Design notes:
  - all matmul operands bf16 (psum f32); inputs shipped bf16, x pre-transposed
    on the host so no on-device transposes are needed
  - LN mean-removal is folded into host-centered weight rows
    ((x-mu) @ W^T == x @ Wc^T with Wc rows centered over the input dim);
    only rstd is computed on device (row-sum matmuls against a ones column),
    and it only scales V
  - q/k rms-norm scales via "broadcast blockdiag" matmuls: the stationary
    replicates 1/gamma^2 across all 128 output columns so each (head, pos)
    squared norm lands on every row of its head's range in one matmul
  - attention is software-pipelined ACROSS (q-chunk, head-pair) chunks: the
    exp stream on the scalar engine is the bottleneck, so each chunk's first
    two score/exp pairs are emitted in the previous chunk's empty score
    slots, scores are emitted ahead of AV within each iteration, softmax
    denominators ride along as a ones-column in the AV stationary, and the
    previous q-chunk's output projection is spread one unit per 4 key-blocks
    across both head-pair chunks so the PE feed rate stays at the exp
    cadence everywhere
"""

import os

import numpy as np

import concourse.bacc as bacc
import concourse.bass as bass
import concourse.mybir as mybir
import concourse.tile as tile
from concourse import bass_utils

try:
    import axon_profile_shim

    axon_profile_shim.install()
except Exception:
    pass

B, N, D = 2, 2048, 1024
H_TOT, DH = 16, 64
HPC = 4  # heads per core
DPC = HPC * DH  # 256 head-dims per core
P = 128
NT = N // P  # 16 seq tiles
KC = D // P  # 8 contraction chunks
NC4 = N // 512  # 4 n-chunks of 512
LN_EPS = 1e-5

F32 = mybir.dt.float32
BF = mybir.dt.bfloat16
AF = mybir.ActivationFunctionType

_CACHE = {}
DEBUG = bool(int(os.environ.get("KERNEL_DEBUG", "0")))


def build():
    nc = bacc.Bacc("TRN2", target_bir_lowering=False, debug=False, num_devices=8)

    xT_d = nc.dram_tensor("xT", [D, N], BF, kind="ExternalInput")
    wq_d = nc.dram_tensor("wqT", [D, DPC], BF, kind="ExternalInput")
    wk_d = nc.dram_tensor("wkT", [D, DPC], BF, kind="ExternalInput")
    wv_d = nc.dram_tensor("wvT", [D, DPC], BF, kind="ExternalInput")
    wo_d = nc.dram_tensor("woT", [DPC, D], BF, kind="ExternalInput")
    bo_d = nc.dram_tensor("bo_bc", [P, D], F32, kind="ExternalInput")
    bd_d = nc.dram_tensor("bd", [P, 2, P], BF, kind="ExternalInput")
    out_d = nc.dram_tensor("out", [N, D], F32, kind="ExternalOutput")
    if DEBUG:
        dbg = {
            nm: nc.dram_tensor(nm, shp, dt, kind="ExternalOutput")
            for nm, (shp, dt) in {
                "dbg_xcT": ([P, KC * N], BF),
                "dbg_qnT": ([P, 2 * N], BF),
                "dbg_knT": ([P, 2 * N], BF),
                "dbg_vsb": ([P, NT * HPC * P], BF),
                "dbg_mrg": ([P, 2 * N], BF),
                "dbg_rstd": ([P, NT], F32),
            }.items()
        }

    with tile.TileContext(nc) as tc:
        with tc.tile_pool(name="outer", bufs=1) as op0:
            xsT = op0.tile([P, KC, N], BF, tag="xsT")
            qnT = op0.tile([P, 2, N], BF, tag="qnT")
            knT = op0.tile([P, 2, N], BF, tag="knT")
            vsb = op0.tile([P, NT, HPC, P], BF, tag="vsb")
            mrg = op0.tile([P, 2, N], BF, tag="mrg")
            rstd_all = op0.tile([P, NT], F32, tag="rstd")
            wq = op0.tile([P, KC, DPC], BF, tag="wq")
            wk = op0.tile([P, KC, DPC], BF, tag="wk")
            wv = op0.tile([P, KC, DPC], BF, tag="wv")
            wo = op0.tile([P, 2, D], BF, tag="wo")
            bo = op0.tile([P, D], F32, tag="bo")
            bd = op0.tile([P, 2, P], BF, tag="bd")

            # denominator ones columns: even-head slots col 64, odd col 0
            for h in range(HPC):
                col = 64 if h % 2 == 0 else 0
                nc.vector.memset(vsb[:, :, h, col : col + 1], 1.0)

            def emit_weight_dmas():
                nc.sync.dma_start(wq, wq_d.ap().rearrange("(c p) m -> p c m", p=P))
                nc.scalar.dma_start(wk, wk_d.ap().rearrange("(c p) m -> p c m", p=P))
                nc.sync.dma_start(wv, wv_d.ap().rearrange("(c p) m -> p c m", p=P))
                nc.scalar.dma_start(wo, wo_d.ap().rearrange("(c p) m -> p c m", p=P))
                nc.sync.dma_start(bo, bo_d.ap())
                nc.scalar.dma_start(bd, bd_d.ap())

            # ---- phase 1: xT streams in by 512-col span; LN mean removal is
            # folded into host-centered weights, so projections consume raw
            # xT.  Per span: row-sum/rowsq-sum matmuls give mu and var for
            # rstd (v scaling only); k/v/q projections + rms-norm scales.
            with (
                tc.tile_pool(name="sqxp", bufs=2) as sqxp,
                tc.tile_pool(name="statp", bufs=2) as statp,
                tc.tile_pool(name="sqp", bufs=3) as sqp,
                tc.tile_pool(name="sdp", bufs=3) as sdp,
                tc.tile_pool(name="rrp", bufs=3) as rrp,
                tc.tile_pool(name="onep", bufs=1) as onep,
                tc.tile_pool(name="ps_raw", bufs=3, space="PSUM") as ps_raw,
                tc.tile_pool(name="ps_n2", bufs=2, space="PSUM") as ps_n2,
                tc.tile_pool(name="ps_v", bufs=2, space="PSUM") as ps_v,
                tc.tile_pool(name="ps_st", bufs=1, space="PSUM") as ps_st,
            ):
                onesD = onep.tile([P, 1], BF)
                nc.vector.memset(onesD, 1.0 / D)
                eps_t = onep.tile([1, 1], F32)
                nc.vector.memset(eps_t, LN_EPS)

                def emit_unit(w_sb, pt, ncn):
                    cs = slice(ncn * 512, (ncn + 1) * 512)
                    raw = ps_raw.tile([P, 512], F32, tag="raw")
                    for dc in range(KC):
                        nc.tensor.matmul(
                            raw,
                            w_sb[:, dc, pt * P : (pt + 1) * P],
                            xsT[:, dc, cs],
                            start=(dc == 0),
                            stop=(dc == KC - 1),
                        )
                    sq = sqp.tile([P, 512], BF, tag="sq")
                    nc.scalar.activation(sq, raw, AF.Square)
                    return raw, sq, cs

                def emit_norm_tail(raw, sq, cs, bd_idx, sqrt_scale, dstT, pt):
                    n2b = ps_n2.tile([P, 512], F32, tag="n2b")
                    nc.tensor.matmul(
                        n2b, bd[:, bd_idx, :], sq, start=True, stop=True
                    )
                    sdt = sdp.tile([P, 512], F32, tag="sdt")
                    nc.scalar.activation(sdt, n2b, AF.Sqrt, scale=sqrt_scale)
                    rr = rrp.tile([P, 512], F32, tag="rr")
                    nc.vector.reciprocal_approx_fast(rr, sdt)
                    nc.vector.tensor_mul(dstT[:, pt, cs], raw, rr)

                def emit_v(st):
                    psv = ps_v.tile([P, DPC], F32, tag="psv")
                    for dc in range(KC):
                        nc.tensor.matmul(
                            psv,
                            xsT[:, dc, st * P : (st + 1) * P],
                            wv[:, dc, :],
                            start=(dc == 0),
                            stop=(dc == KC - 1),
                        )
                    pv = psv.rearrange("p (h e d) -> p h e d", h=2, e=2)
                    nc.vector.tensor_scalar_mul(
                        vsb[:, st, 0:4:2, 0:64],
                        pv[:, :, 0],
                        rstd_all[:, st : st + 1],
                    )
                    nc.vector.tensor_scalar_mul(
                        vsb[:, st, 1:4:2, 64:128],
                        pv[:, :, 1],
                        rstd_all[:, st : st + 1],
                    )

                pend = [None]

                def flush_tail():
                    if pend[0] is not None:
                        emit_norm_tail(*pend[0])
                        pend[0] = None

                # prefetch all xT chunk DMAs up front (xsT is persistent, so
                # the queues stream ahead of the span compute); weights go
                # after span 0's chunks
                for s in range(NC4):
                    cs = slice(s * 512, (s + 1) * 512)
                    for dc in range(KC):
                        qe = (nc.sync, nc.scalar, nc.gpsimd)[dc % 3]
                        qe.dma_start(
                            xsT[:, dc, cs],
                            xT_d.ap()[dc * P : (dc + 1) * P, cs],
                        )
                    if s == 0:
                        emit_weight_dmas()

                for s in range(NC4):
                    cs = slice(s * 512, (s + 1) * 512)
                    # squared xT for the variance row-sums
                    sqx = sqxp.tile([P, KC, 512], BF, tag="sqx")
                    for dc in range(KC):
                        nc.vector.tensor_mul(
                            sqx[:, dc], xsT[:, dc, cs], xsT[:, dc, cs]
                        )
                    # mu row: (1/D) * ones^T @ xT
                    mups = ps_st.tile([1, 512], F32, tag="strow", name="mups")
                    for dc in range(KC):
                        nc.tensor.matmul(
                            mups,
                            onesD,
                            xsT[:, dc, cs],
                            start=(dc == 0),
                            stop=(dc == KC - 1),
                        )
                    raw, sq_, cs_ = emit_unit(wk, 0, s)
                    flush_tail()
                    pend[0] = (raw, sq_, cs_, 1, 1.0 / 64.0, knT, 0)
                    raw, sq_, cs_ = emit_unit(wk, 1, s)
                    flush_tail()
                    pend[0] = (raw, sq_, cs_, 1, 1.0 / 64.0, knT, 1)
                    # E[x^2] row
                    n2xps = ps_st.tile([1, 512], F32, tag="strow", name="n2xps")
                    for dc in range(KC):
                        nc.tensor.matmul(
                            n2xps,
                            onesD,
                            sqx[:, dc],
                            start=(dc == 0),
                            stop=(dc == KC - 1),
                        )
                    flush_tail()
                    # rstd for this span: 1/sqrt(E[x^2] - mu^2 + eps), shipped
                    # to natural [P, tile] layout via an SBUF-shuffle DMA
                    musb = statp.tile([1, 512], F32, tag="musb")
                    nc.vector.tensor_copy(musb, mups)
                    sqmu = statp.tile([1, 512], F32, tag="sqmu")
                    nc.vector.tensor_mul(sqmu, musb, musb)
                    varT = statp.tile([1, 512], F32, tag="varT")
                    nc.vector.tensor_sub(varT, n2xps, sqmu)
                    sdT = statp.tile([1, 512], F32, tag="sdT")
                    nc.scalar.activation(sdT, varT, AF.Sqrt, bias=eps_t)
                    rsT = statp.tile([1, 512], F32, tag="rsT")
                    nc.vector.reciprocal_approx_fast(rsT, sdT)
                    for j in range(4):
                        qe = nc.sync if j % 2 == 0 else nc.scalar
                        qe.dma_start(
                            rstd_all[:, 4 * s + j : 4 * s + j + 1],
                            rsT[:, j * P : (j + 1) * P],
                        )
                    for st in range(4 * s, 4 * s + 4):
                        emit_v(st)
                    raw, sq_, cs_ = emit_unit(wq, 0, s)
                    flush_tail()
                    pend[0] = (raw, sq_, cs_, 0, 1.0, qnT, 0)
                    raw, sq_, cs_ = emit_unit(wq, 1, s)
                    flush_tail()
                    pend[0] = (raw, sq_, cs_, 0, 1.0, qnT, 1)
                flush_tail()

            if DEBUG:
                nc.sync.dma_start(
                    dbg["dbg_xcT"].ap(), xsT.rearrange("p c n -> p (c n)")
                )
                nc.sync.dma_start(dbg["dbg_rstd"].ap(), rstd_all)

            if DEBUG:
                nc.sync.dma_start(
                    dbg["dbg_qnT"].ap(), qnT.rearrange("p c n -> p (c n)")
                )
                nc.sync.dma_start(
                    dbg["dbg_knT"].ap(), knT.rearrange("p c n -> p (c n)")
                )
                nc.sync.dma_start(
                    dbg["dbg_vsb"].ap(), vsb.rearrange("p a b c -> p (a b c)")
                )

            # ---- phase 2: attention + interleaved output projection ----
            with (
                tc.tile_pool(name="expp", bufs=6) as ep,
                tc.tile_pool(name="rec", bufs=4) as rp,
                tc.tile_pool(name="bcp", bufs=4) as bcp,
                tc.tile_pool(name="outp", bufs=4) as outp,
                tc.tile_pool(name="ps_sim", bufs=2, space="PSUM") as ps_sim,
                tc.tile_pool(name="ps_av", bufs=4, space="PSUM") as ps_av,
            ):
                def emit_scores(hp, qs, kt, pss):
                    ks = slice(kt * P, (kt + 1) * P)
                    nc.tensor.matmul(
                        pss[:, 0:512],
                        knT[0:64, hp, ks],
                        qnT[0:64, hp, qs],
                        start=True, stop=True,
                        tile_position=(0, 0),
                    )
                    nc.tensor.matmul(
                        pss[:, 512:1024],
                        knT[64:128, hp, ks],
                        qnT[64:128, hp, qs],
                        start=True, stop=True,
                        tile_position=(64, 0),
                    )

                pending_outproj = [None]
                CHUNKS = [(qc, hp) for qc in range(NC4) for hp in range(2)]
                carry_exs = [None]

                for ci, (qc, hp) in enumerate(CHUNKS):
                    qs = slice(qc * 512, (qc + 1) * 512)
                    if True:
                        h0, h1 = 2 * hp, 2 * hp + 1
                        # scores run 2 kt ahead of AV; the first two pairs of
                        # each chunk are emitted during the previous chunk's
                        # empty score slots (kt14/kt15) so the exp stream
                        # never drains at chunk boundaries
                        if carry_exs[0] is None:
                            exs = [None] * NT
                            for kt in range(2):
                                pss = ps_sim.tile(
                                    [P, 1024], F32, tag="pss", name=f"pssp{kt}"
                                )
                                emit_scores(hp, qs, kt, pss)
                                exs[kt] = ep.tile(
                                    [P, 1024], BF, tag="ex", name=f"exp{kt}"
                                )
                                nc.scalar.activation(exs[kt], pss, AF.Exp)
                        else:
                            exs = carry_exs[0]
                        nxt = CHUNKS[ci + 1] if ci + 1 < len(CHUNKS) else None
                        nexs = [None] * NT if nxt else None
                        pv0 = ps_av.tile([P, 512], F32, tag="pav")
                        pv1 = ps_av.tile([P, 512], F32, tag="pav")
                        for kt in range(NT):
                            # emit scores before AV: both are gated on
                            # exp(kt), but AV can additionally stall on the
                            # pv ring at chunk start without blocking ACT
                            if kt + 2 < NT:
                                pss = ps_sim.tile(
                                    [P, 1024], F32, tag="pss", name=f"pssn{kt}"
                                )
                                emit_scores(hp, qs, kt + 2, pss)
                                nx = ep.tile([P, 1024], BF, tag="ex", name=f"exn{kt}")
                                nc.scalar.activation(nx, pss, AF.Exp)
                                exs[kt + 2] = nx
                            elif nxt is not None:
                                kn = kt + 2 - NT
                                nqc, nhp = nxt
                                nqs = slice(nqc * 512, (nqc + 1) * 512)
                                pss = ps_sim.tile(
                                    [P, 1024], F32, tag="pss", name=f"pssx{kn}"
                                )
                                emit_scores(nhp, nqs, kn, pss)
                                nx = ep.tile([P, 1024], BF, tag="ex", name=f"exx{kn}")
                                nc.scalar.activation(nx, pss, AF.Exp)
                                nexs[kn] = nx
                            ex = exs[kt]
                            nc.tensor.matmul(
                                pv0,
                                vsb[:, kt, h0, :],
                                ex[:, 0:512],
                                start=(kt == 0),
                                stop=(kt == NT - 1),
                            )
                            nc.tensor.matmul(
                                pv1,
                                vsb[:, kt, h1, :],
                                ex[:, 512:1024],
                                start=(kt == 0),
                                stop=(kt == NT - 1),
                            )
                            # previous qc's output projection, spread one
                            # (st, osl) unit per 4 kt across BOTH head-pair
                            # chunks, so no single chunk's PE feed rate drops
                            # below the exp cadence
                            if kt % 4 == 1 and pending_outproj[0]:
                                pending_outproj[0].pop(0)()
                                if not pending_outproj[0]:
                                    pending_outproj[0] = None
                        carry_exs[0] = nexs
                        # merge: mrg = attn_out * 1/den
                        # pv0 (even head): outs rows 0:64, den row 64
                        # pv1 (odd head):  outs rows 64:128, den row 0
                        r0 = rp.tile([P, 512], F32, tag="r0")
                        nc.vector.reciprocal_approx_fast(r0, pv0)
                        # gpsimd broadcast can only source partition 0: shift
                        # the even-head denominator row down.  SWDGE queue —
                        # the HWDGE queues are busy with output writes
                        nc.gpsimd.dma_start(r0[0:1], r0[64:65])
                        bc0 = bcp.tile([P, 512], F32, tag="bc0")
                        nc.gpsimd.partition_broadcast(bc0, r0[0:1])
                        nc.vector.tensor_mul(
                            mrg[0:64, hp, qs], pv0[0:64], bc0[0:64]
                        )
                        r1 = rp.tile([P, 512], F32, tag="r1")
                        nc.vector.reciprocal_approx_fast(r1[0:1], pv1[0:1])
                        bc1 = bcp.tile([P, 512], F32, tag="bc1")
                        nc.gpsimd.partition_broadcast(bc1, r1[0:1])
                        nc.vector.tensor_mul(
                            mrg[64:128, hp, qs], pv1[64:128], bc1[64:128]
                        )

                    # output projection for this qc (both head-pairs merged);
                    # deferred + spread into the next chunk's kt loop
                    def make_outproj_unit(st, ncn):
                        def emit():
                            ss = slice(st * P, (st + 1) * P)
                            osl = slice(ncn * 512, (ncn + 1) * 512)
                            psf = ps_av.tile(
                                [P, 512], F32, tag="pav", name=f"psf{st}_{ncn}"
                            )
                            for pt in range(2):
                                nc.tensor.matmul(
                                    psf,
                                    mrg[:, pt, ss],
                                    wo[:, pt, osl],
                                    start=(pt == 0),
                                    stop=(pt == 1),
                                )
                            ot = outp.tile(
                                [P, 512], F32, tag="ot", name=f"ot{st}_{ncn}"
                            )
                            nc.vector.tensor_add(ot, psf, bo[:, osl])
                            qe = nc.sync if (st + ncn) % 2 == 0 else nc.scalar
                            qe.dma_start(out_d.ap()[ss, osl], ot)

                        return emit

                    if hp == 1:
                        pending_outproj[0] = [
                            make_outproj_unit(qc * 4 + sti, ncn)
                            for sti in range(4)
                            for ncn in range(2)
                        ]
                for u in pending_outproj[0]:
                    u()

            if DEBUG:
                nc.sync.dma_start(
                    dbg["dbg_mrg"].ap(), mrg.rearrange("p c n -> p (c n)")
                )

    nc.compile()
    return nc


def _prep_core_inputs(inputs, c):
    import ml_dtypes

    b, g = c // 4, c % 4
    S = slice(DPC * g, DPC * (g + 1))
    x = np.ascontiguousarray(np.asarray(inputs["x"], np.float32)[b])
    lng = np.asarray(inputs["ln_gamma"], np.float32)
    lnb = np.asarray(inputs["ln_beta"], np.float32)
    qg = np.asarray(inputs["q_gamma"], np.float32)
    kg = np.asarray(inputs["k_gamma"], np.float32)
    if np.abs(lnb).max() > 0:
        raise NotImplementedError("nonzero ln_beta not supported by this kernel")
    bf16 = ml_dtypes.bfloat16
    w_q = np.asarray(inputs["w_q"], np.float32)[S] * lng[None, :]
    w_k = np.asarray(inputs["w_k"], np.float32)[S] * lng[None, :]
    w_v = np.asarray(inputs["w_v"], np.float32)[S] * lng[None, :]
    w_q = w_q * np.tile(qg, HPC)[:, None]
    w_k = w_k * np.tile(kg, HPC)[:, None]
    # LN mean-removal folded into the weights: (x - mu) @ W^T == x @ Wc^T
    # when each weight row is centered over the input dimension
    w_q = w_q - w_q.mean(axis=1, keepdims=True)
    w_k = w_k - w_k.mean(axis=1, keepdims=True)
    w_v = w_v - w_v.mean(axis=1, keepdims=True)
    w_o = np.asarray(inputs["w_o"], np.float32)[:, S]
    b_o = np.asarray(inputs["b_o"], np.float32)
    bo_bc = np.tile((b_o if g == 0 else np.zeros_like(b_o))[None, :], (P, 1))

    # broadcast blockdiag weights: bd[p, j, m] = 1/gamma[p%64]^2 when
    # partition p and output row m fall in the same head half; the resulting
    # [128, 512] matmul output carries each head's squared norm on every row
    # of that head's range (broadcast fused into the reduction).
    bd = np.zeros((P, 2, P), np.float32)
    for j, g_ in ((0, qg), (1, kg)):
        inv = 1.0 / np.maximum(g_**2, 1e-30)
        bd[0:64, j, 0:64] = inv[:, None]
        bd[64:128, j, 64:128] = inv[:, None]

    return {
        "xT": np.ascontiguousarray(x.T).astype(bf16),
        "wqT": np.ascontiguousarray(w_q.T).astype(bf16),
        "wkT": np.ascontiguousarray(w_k.T).astype(bf16),
        "wvT": np.ascontiguousarray(w_v.T).astype(bf16),
        "woT": np.ascontiguousarray(w_o.T).astype(bf16),
        "bo_bc": bo_bc,
        "bd": bd.astype(bf16),
    }


def kernel(**inputs):
    if "nc" not in _CACHE:
        _CACHE["nc"] = build()
    nc = _CACHE["nc"]
    in_maps = [_prep_core_inputs(inputs, c) for c in range(8)]
    res = bass_utils.run_bass_kernel_spmd(
        nc,
        in_maps,
        core_ids=list(range(8)),
        trace=bool(int(os.environ.get("KERNEL_TRACE", "0"))),
    )
    _CACHE["last_result"] = res
    out = np.zeros((B, N, D), np.float32)
    for c in range(8):
        out[c // 4] += res.results[c]["out"]
    return out
